# revision 12
# baseline (speedup 1.0000x reference)
"""Trainium2 Bass kernel: per-pixel 5x5 kernel application (KPN-style).

    out[b,c,y,x] = sum_{i,j} softmax(kernels[b,:,y,x])[i*5+j]
                   * zpad(data)[b,c,y+i,x+j]          (i,j in 0..4, r=2)

Sharding (8 NeuronCores, pure data parallel, no collectives):
    core = (b, H-half): 4 batches x 2 row-bands of 360 rows.

Band layout: partition p = x-band of 10 columns (128 bands x 10 = 1280).
Rows live in the free dimension, so BOTH the di (row) and dj (col) tap
shifts become free-dim AP offsets -- no shift matrices, no partition
crossing.  Per accumulation tile (15/30-row warmup tiles, then 45 rows):

    - ACT: E = exp(logits) in one op (fp16).
    - DVE: per (di, c): two batched products q = E * D (dj in the AP's
      outer free dim; even dj read D0, odd dj read D1 = D0 shifted one
      element so operands stay 4-byte aligned for the 2x DVE mode).
      A few odd-dj product ops run on GpSimd instead (tensor_tensor is
      2x_1P on DVE, so the two engines don't contend for SBUF ports).
    - PE:  identity-lhsT matmuls accumulate the 75 tap planes and the
      25 exp planes into 4 PSUM banks (start/stop per bank).  The
      stationary operand never changes, so the PE stays warm.
    - normalize: R32 = reciprocal_approx_fast(sumexp) (DVE), R16 cast
      (GpSimd), P copied PSUM->SBUF fp16 (ACT), out = P * R (DVE, 2x).

DMA: everything big rides SWDGE (gpsimd) so descriptors spray across
all 16 SDMA engines with one contiguous run per partition.  The first
kE load is emitted BEFORE the D tiles so nothing queues ahead of the
exp->product critical path at startup; D0/D1 are split in two row
chunks each.  Stores ride the HWDGE rings (engines 0-3, otherwise idle).

kernel(**inputs) takes the FULL inputs and returns the FULL output.
"""

import numpy as np
from numpy.lib.stride_tricks import sliding_window_view

B, C, H, W, KW = 4, 3, 720, 1280, 5
NCORES = 8
HS = H // 2            # 360 output rows per shard
NB = 128               # x-bands (partitions)
BW = 10                # band width (output columns per partition)
DW = 14                # data band width incl. 2+2 halo columns
DR = HS + 4            # 364 data rows incl. 2+2 halo rows
TAPS = KW * KW

# accumulation tiles: small warmup tiles shorten the startup pipeline,
# a small final tile shortens the drain
TILES = ([(0, 15), (15, 30)] + [(45 * k, 45) for k in range(1, 7)]
         + [(315, 30), (345, 15)])
DSPLIT = 188           # D tiles load in rows [0,188) + [188,364)

# host tap order: within each di group, dj = 0,2,4,1,3 (even-first so
# the even/odd product APs are plain slices)
DJ_ORDER = [0, 2, 4, 1, 3]
TAP_PERM = [di * KW + dj for di in range(KW) for dj in DJ_ORDER]

# odd-dj product ops computed on GpSimd instead of DVE
GP_QO = {(1, 2), (2, 2), (3, 2)}
GP_RCAST = False

_CACHE = {}


def _build_program():
    import concourse.bacc as bacc
    import concourse.mybir as mybir
    from concourse.bass import AP
    from concourse import tile

    f32 = mybir.dt.float32
    f16 = mybir.dt.float16

    nc = bacc.Bacc(
        "TRN2",
        target_bir_lowering=False,
        debug=False,
        enable_asserts=False,
        num_devices=NCORES,
    )
    d_ke = nc.dram_tensor("ke", [NB, HS, TAPS * BW], f16, kind="ExternalInput")
    d_db0 = nc.dram_tensor("db0", [NB, DR, C, DW], f16, kind="ExternalInput")
    d_db1 = nc.dram_tensor("db1", [NB, DR, C, DW], f16, kind="ExternalInput")
    d_out = nc.dram_tensor("out", [NB, HS, C * BW], f16, kind="ExternalOutput")

    d_id = nc.inline_tensor(np.eye(NB, dtype=np.float16), "ident")

    KE_ROW = TAPS * BW          # 250 elems per (band, row)
    D_ROW = C * DW              # 42 elems per (band, row)

    with tile.TileContext(nc) as tc:
        with tc.tile_pool(name="const", bufs=1) as cpool, \
             tc.tile_pool(name="kt", bufs=2) as kpool, \
             tc.tile_pool(name="et", bufs=2) as epool, \
             tc.tile_pool(name="dt", bufs=1) as dpool, \
             tc.tile_pool(name="qt", bufs=4) as qpool, \
             tc.tile_pool(name="rt", bufs=2) as rpool, \
             tc.tile_pool(name="ot", bufs=2) as opool, \
             tc.tile_pool(name="ps", bufs=2, space="PSUM") as ppool:

            id_sb = cpool.tile([NB, NB], f16)
            nc.sync.dma_start(out=id_sb[:], in_=d_id.ap())

            D0 = dpool.tile([NB, DR, C, DW], f16, tag="d0")
            D1 = dpool.tile([NB, DR, C, DW], f16, tag="d1")

            st_eng = [nc.sync, nc.scalar]

            # deferred epilogue: normalize+store of tile t runs while
            # tile t+1's products stream, so the DVE never stalls
            # waiting for the PE to drain the accumulation group
            pending = []

            def epilogue():
                if not pending:
                    return
                t, r0, nr, pacc = pending.pop(0)
                fd = nr * BW
                R32 = rpool.tile([NB, nr, BW], f32, tag="r32")
                nc.vector.reciprocal_approx_fast(
                    out=R32[:].rearrange("p r x -> p (r x)"),
                    in_=pacc[:, 3, 0:fd],
                )
                R16 = rpool.tile([NB, nr, BW], f16, tag="r16")
                if GP_RCAST:
                    nc.gpsimd.tensor_scalar_mul(
                        R16[:].rearrange("p r x -> p (r x)"),
                        R32[:].rearrange("p r x -> p (r x)"),
                        1.0,
                    )
                else:
                    nc.vector.tensor_copy(
                        R16[:].rearrange("p r x -> p (r x)"),
                        R32[:].rearrange("p r x -> p (r x)"),
                    )
                # P: PSUM -> SBUF fp16 on ACT (frees DVE from the slow
                # 1x PSUM-source read)
                Pst = opool.tile([NB, nr, C, BW], f16, tag="pst")
                p_view = AP(
                    pacc[:].tensor, 0,
                    [[4 * 512, NB], [BW, nr], [512, C], [1, BW]],
                )
                nc.scalar.activation(
                    Pst[:], p_view, mybir.ActivationFunctionType.Copy,
                )
                outst = opool.tile([NB, nr, C, BW], f16, tag="o")
                r_bc = AP(
                    R16[:].tensor, 0,
                    [[nr * BW, NB], [BW, nr], [0, C], [1, BW]],
                )
                nc.vector.tensor_tensor(
                    outst[:], Pst[:], r_bc, mybir.AluOpType.mult)
                st_eng[t % 2].dma_start(
                    out=d_out.ap()[:, r0:r0 + nr],
                    in_=outst[:].rearrange("p r c x -> p r (c x)"),
                )

            for t, (r0, nr) in enumerate(TILES):
                fd = nr * BW
                kE = kpool.tile([NB, nr, KE_ROW], f16, tag="ke")
                nc.gpsimd.dma_start(
                    out=kE[:], in_=d_ke.ap()[:, r0:r0 + nr]
                )
                # D chunks ride the same SWDGE queue, behind the kE
                # loads whose consumers they race
                if t == 0:
                    nc.gpsimd.dma_start(
                        out=D0[:, 0:DSPLIT], in_=d_db0.ap()[:, 0:DSPLIT])
                    nc.gpsimd.dma_start(
                        out=D1[:, 0:DSPLIT], in_=d_db1.ap()[:, 0:DSPLIT])
                elif t == 2:
                    # rows >= 188 are first needed by tile 4; keep these
                    # big chunks behind tile 2's kE load in the SWDGE queue
                    nc.gpsimd.dma_start(
                        out=D0[:, DSPLIT:DR], in_=d_db0.ap()[:, DSPLIT:DR])
                    nc.gpsimd.dma_start(
                        out=D1[:, DSPLIT:DR], in_=d_db1.ap()[:, DSPLIT:DR])

                E = epool.tile([NB, nr, TAPS, BW], f16, tag="e")
                nc.scalar.activation(
                    E[:].rearrange("p r t x -> p (r t x)"),
                    kE[:].rearrange("p r k -> p (r k)"),
                    mybir.ActivationFunctionType.Exp,
                )
                eap = E[:]

                pacc = ppool.tile([NB, 4, 512], f32, tag="pacc")

                for di in range(KW):
                    # products: q = E * D, dj batched in the outer free dim
                    qes, qos = [], []
                    for c in range(C):
                        qe = qpool.tile([NB, 3, nr, BW], f16, tag="qe")
                        qo = qpool.tile([NB, 2, nr, BW], f16, tag="qo")
                        e_even = AP(
                            eap.tensor, (KW * di) * BW,
                            [[nr * KE_ROW, NB], [BW, 3], [KE_ROW, nr], [1, BW]],
                        )
                        e_odd = AP(
                            eap.tensor, (KW * di + 3) * BW,
                            [[nr * KE_ROW, NB], [BW, 2], [KE_ROW, nr], [1, BW]],
                        )
                        doff = (r0 + di) * D_ROW + c * DW
                        d_even = AP(
                            D0[:].tensor, doff,
                            [[DR * D_ROW, NB], [2, 3], [D_ROW, nr], [1, BW]],
                        )
                        d_odd = AP(
                            D1[:].tensor, doff,
                            [[DR * D_ROW, NB], [2, 2], [D_ROW, nr], [1, BW]],
                        )
                        nc.vector.tensor_tensor(
                            qe[:], e_even, d_even, mybir.AluOpType.mult)
                        qo_eng = nc.gpsimd if (di, c) in GP_QO else nc.vector
                        qo_eng.tensor_tensor(
                            qo[:], e_odd, d_odd, mybir.AluOpType.mult)
                        qes.append(qe)
                        qos.append(qo)
                    # sumexp: 5 identity matmuls straight off E (no DVE dep)
                    for k in range(KW):
                        tp = KW * di + k
                        nc.tensor.matmul(
                            out=pacc[:, 3, 0:fd],
                            lhsT=id_sb[:],
                            rhs=eap[:, :, tp, :],
                            start=(tp == 0),
                            stop=(tp == TAPS - 1),
                        )
                    # tap accumulation
                    for c in range(C):
                        for k in range(3):
                            nc.tensor.matmul(
                                out=pacc[:, c, 0:fd],
                                lhsT=id_sb[:],
                                rhs=qes[c][:, k],
                                start=(di == 0 and k == 0),
                                stop=False,
                            )
                        for k in range(2):
                            nc.tensor.matmul(
                                out=pacc[:, c, 0:fd],
                                lhsT=id_sb[:],
                                rhs=qos[c][:, k],
                                start=False,
                                stop=(di == KW - 1 and k == 1),
                            )

                pending.append((t, r0, nr, pacc))
                if len(pending) >= 2:
                    epilogue()

            while pending:
                epilogue()

    nc.compile()
    return nc


def get_program():
    if "nc" not in _CACHE:
        _CACHE["nc"] = _build_program()
    return _CACHE["nc"]


def make_shards(data: np.ndarray, kernels: np.ndarray):
    """Full inputs -> per-core input maps (band layout, fp16)."""
    data = np.asarray(data, dtype=np.float32)
    kernels = np.asarray(kernels, dtype=np.float32)

    kf = kernels[:, TAP_PERM].astype(np.float16)      # [B, 25, H, W]
    dpad = np.zeros((B, C, H + 4, W + 6), dtype=np.float16)
    dpad[:, :, 2:H + 2, 2:W + 2] = data

    in_maps = []
    for core in range(NCORES):
        b, hh = divmod(core, 2)
        r0 = hh * HS
        ks = kf[b, :, r0:r0 + HS, :]                  # [25, 360, 1280]
        ke = np.ascontiguousarray(
            ks.reshape(TAPS, HS, NB, BW).transpose(2, 1, 0, 3)
        ).reshape(NB, HS, TAPS * BW)
        dsl = dpad[b, :, r0:r0 + DR, :]               # [3, 364, 1286]
        win = sliding_window_view(dsl, DW, axis=2)    # [3, 364, 1273, 14]
        db0 = np.ascontiguousarray(
            win[:, :, 0:NB * BW:BW].transpose(2, 1, 0, 3))   # [128,364,3,14]
        db1 = np.ascontiguousarray(
            win[:, :, 1:NB * BW + 1:BW].transpose(2, 1, 0, 3))
        in_maps.append({"ke": ke, "db0": db0, "db1": db1})
    return in_maps


def unshard_out(arr: np.ndarray) -> np.ndarray:
    """Per-core out [NB, HS, C*BW] fp16 -> [C, HS, W] f32."""
    o = arr.reshape(NB, HS, C, BW).transpose(2, 1, 0, 3)
    return np.ascontiguousarray(o).reshape(C, HS, W).astype(np.float32)


def assemble(results) -> np.ndarray:
    out = np.empty((B, C, H, W), dtype=np.float32)
    for core in range(NCORES):
        b, hh = divmod(core, 2)
        out[b, :, hh * HS:(hh + 1) * HS, :] = unshard_out(results[core]["out"])
    return out


def kernel(data: np.ndarray, kernels: np.ndarray) -> np.ndarray:
    from concourse.bass_utils import run_bass_kernel_spmd

    nc = get_program()
    in_maps = make_shards(data, kernels)
    res = run_bass_kernel_spmd(nc, in_maps, list(range(NCORES)))
    return assemble(res.results)


if __name__ == "__main__":
    get_program()
    print("program built OK")


# revision 15
# speedup vs baseline: 1.2210x; 1.2210x over previous
"""Trainium2 Bass kernel: per-pixel 5x5 kernel application (KPN-style).

    out[b,c,y,x] = sum_{i,j} softmax(kernels[b,:,y,x])[i*5+j]
                   * zpad(data)[b,c,y+i,x+j]          (i,j in 0..4, r=2)

Sharding (8 NeuronCores, pure data parallel, no collectives):
    core = (b, H-half): 4 batches x 2 row-bands of 360 rows.

Band layout: partition p = x-band of 10 columns (128 bands x 10 = 1280).
Rows live in the free dimension, so BOTH the di (row) and dj (col) tap
shifts become free-dim AP offsets -- no shift matrices, no partition
crossing.  Per accumulation tile (15/30-row warmup tiles, then 45 rows):

    - ACT: E = exp(logits) in one op (fp16).
    - DVE: per (di, c): two batched products q = E * D (dj in the AP's
      outer free dim; even dj read D0, odd dj read D1 = D0 shifted one
      element so operands stay 4-byte aligned for the 2x DVE mode).
      A few odd-dj product ops run on GpSimd instead (tensor_tensor is
      2x_1P on DVE, so the two engines don't contend for SBUF ports).
    - PE:  identity-lhsT matmuls accumulate the 75 tap planes and the
      25 exp planes into 4 PSUM banks (start/stop per bank).  The
      stationary operand never changes, so the PE stays warm.
    - normalize: R32 = reciprocal_approx_fast(sumexp) (DVE), R16 cast
      (GpSimd), P copied PSUM->SBUF fp16 (ACT), out = P * R (DVE, 2x).

DMA: everything big rides SWDGE (gpsimd) so descriptors spray across
all 16 SDMA engines with one contiguous run per partition.  The first
kE load is emitted BEFORE the D tiles so nothing queues ahead of the
exp->product critical path at startup; D0/D1 are split in two row
chunks each.  Stores ride the HWDGE rings (engines 0-3, otherwise idle).

kernel(**inputs) takes the FULL inputs and returns the FULL output.
"""

import numpy as np
from numpy.lib.stride_tricks import sliding_window_view

B, C, H, W, KW = 4, 3, 720, 1280, 5
NCORES = 8
HS = H // 2            # 360 output rows per shard
NB = 128               # x-bands (partitions)
BW = 10                # band width (output columns per partition)
DW = 14                # data band width incl. 2+2 halo columns
DR = HS + 4            # 364 data rows incl. 2+2 halo rows
TAPS = KW * KW

# accumulation tiles: small warmup tiles shorten the startup pipeline,
# a small final tile shortens the drain
TILES = ([(0, 15), (15, 30)] + [(45 * k, 45) for k in range(1, 7)]
         + [(315, 30), (345, 15)])
DSPLIT = 188           # D tiles load in rows [0,188) + [188,364)

# host tap order: within each di group, dj = 0,2,4,1,3 (even-first so
# the even/odd product APs are plain slices)
DJ_ORDER = [0, 2, 4, 1, 3]
TAP_PERM = [di * KW + dj for di in range(KW) for dj in DJ_ORDER]

# odd-dj product ops computed on GpSimd instead of DVE.  Measured: a
# GpSimd TT of this shape takes ~3us (6x the DVE cost) and slows
# concurrent DVE ops ~10% -- never worth it.  Keep empty.
GP_QO = set()
GP_RCAST = False

_CACHE = {}


def _build_program():
    import concourse.bacc as bacc
    import concourse.mybir as mybir
    from concourse.bass import AP
    from concourse import tile

    f32 = mybir.dt.float32
    f16 = mybir.dt.float16

    nc = bacc.Bacc(
        "TRN2",
        target_bir_lowering=False,
        debug=False,
        enable_asserts=False,
        num_devices=NCORES,
    )
    d_ke = nc.dram_tensor("ke", [NB, HS, TAPS * BW], f16, kind="ExternalInput")
    d_db0 = nc.dram_tensor("db0", [NB, DR, C, DW], f16, kind="ExternalInput")
    d_db1 = nc.dram_tensor("db1", [NB, DR, C, DW], f16, kind="ExternalInput")
    d_out = nc.dram_tensor("out", [NB, HS, C * BW], f16, kind="ExternalOutput")

    d_id = nc.inline_tensor(np.eye(NB, dtype=np.float16), "ident")

    KE_ROW = TAPS * BW          # 250 elems per (band, row)
    D_ROW = C * DW              # 42 elems per (band, row)

    with tile.TileContext(nc) as tc:
        with tc.tile_pool(name="const", bufs=1) as cpool, \
             tc.tile_pool(name="kt", bufs=2) as kpool, \
             tc.tile_pool(name="et", bufs=2) as epool, \
             tc.tile_pool(name="dt", bufs=1) as dpool, \
             tc.tile_pool(name="qt", bufs=4) as qpool, \
             tc.tile_pool(name="rt", bufs=2) as rpool, \
             tc.tile_pool(name="ot", bufs=2) as opool, \
             tc.tile_pool(name="ps", bufs=2, space="PSUM") as ppool:

            id_sb = cpool.tile([NB, NB], f16)
            nc.sync.dma_start(out=id_sb[:], in_=d_id.ap())

            D0 = dpool.tile([NB, DR, C, DW], f16, tag="d0")
            D1 = dpool.tile([NB, DR, C, DW], f16, tag="d1")

            st_eng = [nc.sync, nc.scalar]

            # deferred epilogue: normalize+store of tile t runs while
            # tile t+1's products stream, so the DVE never stalls
            # waiting for the PE to drain the accumulation group
            pending = []

            def epilogue():
                if not pending:
                    return
                t, r0, nr, pacc = pending.pop(0)
                fd = nr * BW
                R32 = rpool.tile([NB, nr, BW], f32, tag="r32")
                nc.vector.reciprocal_approx_fast(
                    out=R32[:].rearrange("p r x -> p (r x)"),
                    in_=pacc[:, 3, 0:fd],
                )
                outst = opool.tile([NB, nr, C, BW], f16, tag="o")
                p_view = AP(
                    pacc[:].tensor, 0,
                    [[4 * 512, NB], [BW, nr], [512, C], [1, BW]],
                )
                r_bc = AP(
                    R32[:].tensor, 0,
                    [[nr * BW, NB], [BW, nr], [0, C], [1, BW]],
                )
                nc.vector.tensor_tensor(
                    outst[:], p_view, r_bc, mybir.AluOpType.mult)
                st_eng[t % 2].dma_start(
                    out=d_out.ap()[:, r0:r0 + nr],
                    in_=outst[:].rearrange("p r c x -> p r (c x)"),
                )

            for t, (r0, nr) in enumerate(TILES):
                fd = nr * BW
                kE = kpool.tile([NB, nr, KE_ROW], f16, tag="ke")
                nc.gpsimd.dma_start(
                    out=kE[:], in_=d_ke.ap()[:, r0:r0 + nr]
                )
                # D chunks ride the same SWDGE queue, behind the kE
                # loads whose consumers they race
                if t == 0:
                    nc.gpsimd.dma_start(
                        out=D0[:, 0:DSPLIT], in_=d_db0.ap()[:, 0:DSPLIT])
                    nc.gpsimd.dma_start(
                        out=D1[:, 0:DSPLIT], in_=d_db1.ap()[:, 0:DSPLIT])
                elif t == 2:
                    # rows >= 188 are first needed by tile 4; keep these
                    # big chunks behind tile 2's kE load in the SWDGE queue
                    nc.gpsimd.dma_start(
                        out=D0[:, DSPLIT:DR], in_=d_db0.ap()[:, DSPLIT:DR])
                    nc.gpsimd.dma_start(
                        out=D1[:, DSPLIT:DR], in_=d_db1.ap()[:, DSPLIT:DR])

                E = epool.tile([NB, nr, TAPS, BW], f16, tag="e")
                nc.scalar.activation(
                    E[:].rearrange("p r t x -> p (r t x)"),
                    kE[:].rearrange("p r k -> p (r k)"),
                    mybir.ActivationFunctionType.Exp,
                )
                eap = E[:]

                pacc = ppool.tile([NB, 4, 512], f32, tag="pacc")

                for di in range(KW):
                    # products: q = E * D, dj batched in the outer free dim
                    qes, qos = [], []
                    for c in range(C):
                        qe = qpool.tile([NB, 3, nr, BW], f16, tag="qe")
                        qo = qpool.tile([NB, 2, nr, BW], f16, tag="qo")
                        e_even = AP(
                            eap.tensor, (KW * di) * BW,
                            [[nr * KE_ROW, NB], [BW, 3], [KE_ROW, nr], [1, BW]],
                        )
                        e_odd = AP(
                            eap.tensor, (KW * di + 3) * BW,
                            [[nr * KE_ROW, NB], [BW, 2], [KE_ROW, nr], [1, BW]],
                        )
                        doff = (r0 + di) * D_ROW + c * DW
                        d_even = AP(
                            D0[:].tensor, doff,
                            [[DR * D_ROW, NB], [2, 3], [D_ROW, nr], [1, BW]],
                        )
                        d_odd = AP(
                            D1[:].tensor, doff,
                            [[DR * D_ROW, NB], [2, 2], [D_ROW, nr], [1, BW]],
                        )
                        nc.vector.tensor_tensor(
                            qe[:], e_even, d_even, mybir.AluOpType.mult)
                        qo_eng = nc.gpsimd if (di, c) in GP_QO else nc.vector
                        qo_eng.tensor_tensor(
                            qo[:], e_odd, d_odd, mybir.AluOpType.mult)
                        qes.append(qe)
                        qos.append(qo)
                    # sumexp: 5 identity matmuls straight off E (no DVE dep)
                    for k in range(KW):
                        tp = KW * di + k
                        nc.tensor.matmul(
                            out=pacc[:, 3, 0:fd],
                            lhsT=id_sb[:],
                            rhs=eap[:, :, tp, :],
                            start=(tp == 0),
                            stop=(tp == TAPS - 1),
                        )
                    # tap accumulation
                    for c in range(C):
                        for k in range(3):
                            nc.tensor.matmul(
                                out=pacc[:, c, 0:fd],
                                lhsT=id_sb[:],
                                rhs=qes[c][:, k],
                                start=(di == 0 and k == 0),
                                stop=False,
                            )
                        for k in range(2):
                            nc.tensor.matmul(
                                out=pacc[:, c, 0:fd],
                                lhsT=id_sb[:],
                                rhs=qos[c][:, k],
                                start=False,
                                stop=(di == KW - 1 and k == 1),
                            )
                    if di == 0:
                        # previous tile's normalize+store: by now its
                        # accumulation group has drained, so the DVE
                        # doesn't stall on the PSUM-stop semaphore
                        epilogue()

                pending.append((t, r0, nr, pacc))

            while pending:
                epilogue()

    nc.compile()
    return nc


def get_program():
    if "nc" not in _CACHE:
        _CACHE["nc"] = _build_program()
    return _CACHE["nc"]


def make_shards(data: np.ndarray, kernels: np.ndarray):
    """Full inputs -> per-core input maps (band layout, fp16)."""
    data = np.asarray(data, dtype=np.float32)
    kernels = np.asarray(kernels, dtype=np.float32)

    kf = kernels[:, TAP_PERM].astype(np.float16)      # [B, 25, H, W]
    dpad = np.zeros((B, C, H + 4, W + 6), dtype=np.float16)
    dpad[:, :, 2:H + 2, 2:W + 2] = data

    in_maps = []
    for core in range(NCORES):
        b, hh = divmod(core, 2)
        r0 = hh * HS
        ks = kf[b, :, r0:r0 + HS, :]                  # [25, 360, 1280]
        ke = np.ascontiguousarray(
            ks.reshape(TAPS, HS, NB, BW).transpose(2, 1, 0, 3)
        ).reshape(NB, HS, TAPS * BW)
        dsl = dpad[b, :, r0:r0 + DR, :]               # [3, 364, 1286]
        win = sliding_window_view(dsl, DW, axis=2)    # [3, 364, 1273, 14]
        db0 = np.ascontiguousarray(
            win[:, :, 0:NB * BW:BW].transpose(2, 1, 0, 3))   # [128,364,3,14]
        db1 = np.ascontiguousarray(
            win[:, :, 1:NB * BW + 1:BW].transpose(2, 1, 0, 3))
        in_maps.append({"ke": ke, "db0": db0, "db1": db1})
    return in_maps


def unshard_out(arr: np.ndarray) -> np.ndarray:
    """Per-core out [NB, HS, C*BW] fp16 -> [C, HS, W] f32."""
    o = arr.reshape(NB, HS, C, BW).transpose(2, 1, 0, 3)
    return np.ascontiguousarray(o).reshape(C, HS, W).astype(np.float32)


def assemble(results) -> np.ndarray:
    out = np.empty((B, C, H, W), dtype=np.float32)
    for core in range(NCORES):
        b, hh = divmod(core, 2)
        out[b, :, hh * HS:(hh + 1) * HS, :] = unshard_out(results[core]["out"])
    return out


def kernel(data: np.ndarray, kernels: np.ndarray) -> np.ndarray:
    from concourse.bass_utils import run_bass_kernel_spmd

    nc = get_program()
    in_maps = make_shards(data, kernels)
    res = run_bass_kernel_spmd(nc, in_maps, list(range(NCORES)))
    return assemble(res.results)


if __name__ == "__main__":
    get_program()
    print("program built OK")


# revision 17
# speedup vs baseline: 1.2290x; 1.0065x over previous
"""Trainium2 Bass kernel: per-pixel 5x5 kernel application (KPN-style).

    out[b,c,y,x] = sum_{i,j} softmax(kernels[b,:,y,x])[i*5+j]
                   * zpad(data)[b,c,y+i,x+j]          (i,j in 0..4, r=2)

Sharding (8 NeuronCores, pure data parallel, no collectives):
    core = (b, H-half): 4 batches x 2 row-bands of 360 rows.

Band layout: partition p = x-band of 10 columns (128 bands x 10 = 1280).
Rows live in the free dimension, so BOTH the di (row) and dj (col) tap
shifts become free-dim AP offsets -- no shift matrices, no partition
crossing.  Per accumulation tile (15/30-row warmup tiles, then 45 rows):

    - ACT: E = exp(logits) in one op (fp16).
    - DVE: per (di, c): two batched products q = E * D (dj in the AP's
      outer free dim; even dj read D0, odd dj read D1 = D0 shifted one
      element so operands stay 4-byte aligned for the 2x DVE mode).
      A few odd-dj product ops run on GpSimd instead (tensor_tensor is
      2x_1P on DVE, so the two engines don't contend for SBUF ports).
    - PE:  identity-lhsT matmuls accumulate the 75 tap planes and the
      25 exp planes into 4 PSUM banks (start/stop per bank).  The
      stationary operand never changes, so the PE stays warm.
    - normalize: R32 = reciprocal_approx_fast(sumexp) (DVE), R16 cast
      (GpSimd), P copied PSUM->SBUF fp16 (ACT), out = P * R (DVE, 2x).

DMA: everything big rides SWDGE (gpsimd) so descriptors spray across
all 16 SDMA engines with one contiguous run per partition.  The first
kE load is emitted BEFORE the D tiles so nothing queues ahead of the
exp->product critical path at startup; D0/D1 are split in two row
chunks each.  Stores ride the HWDGE rings (engines 0-3, otherwise idle).

kernel(**inputs) takes the FULL inputs and returns the FULL output.
"""

import numpy as np
from numpy.lib.stride_tricks import sliding_window_view

B, C, H, W, KW = 4, 3, 720, 1280, 5
NCORES = 8
HS = H // 2            # 360 output rows per shard
NB = 128               # x-bands (partitions)
BW = 10                # band width (output columns per partition)
DW = 14                # data band width incl. 2+2 halo columns
DR = HS + 4            # 364 data rows incl. 2+2 halo rows
TAPS = KW * KW

# accumulation tiles: small warmup tiles shorten the startup pipeline,
# a small final tile shortens the drain
TILES = ([(0, 15), (15, 30)] + [(45 * k, 45) for k in range(1, 7)]
         + [(315, 30), (345, 15)])
DSPLIT = 188           # D tiles load in rows [0,188) + [188,364)

# host tap order: within each di group, dj = 0,2,4,1,3 (even-first so
# the even/odd product APs are plain slices)
DJ_ORDER = [0, 2, 4, 1, 3]
TAP_PERM = [di * KW + dj for di in range(KW) for dj in DJ_ORDER]

# odd-dj product ops computed on GpSimd instead of DVE.  Measured: a
# GpSimd TT of this shape takes ~3us (6x the DVE cost) and slows
# concurrent DVE ops ~10% -- never worth it.  Keep empty.
GP_QO = set()
GP_RCAST = False

_CACHE = {}


def _build_program():
    import concourse.bacc as bacc
    import concourse.mybir as mybir
    from concourse.bass import AP
    from concourse import tile

    f32 = mybir.dt.float32
    f16 = mybir.dt.float16

    nc = bacc.Bacc(
        "TRN2",
        target_bir_lowering=False,
        debug=False,
        enable_asserts=False,
        num_devices=NCORES,
    )
    d_ke = nc.dram_tensor("ke", [NB, HS, TAPS * BW], f16, kind="ExternalInput")
    d_db0 = nc.dram_tensor("db0", [NB, DR, C, DW], f16, kind="ExternalInput")
    d_db1 = nc.dram_tensor("db1", [NB, DR, C, DW], f16, kind="ExternalInput")
    d_out = nc.dram_tensor("out", [NB, HS, C * BW], f16, kind="ExternalOutput")

    d_id = nc.inline_tensor(np.eye(NB, dtype=np.float16), "ident")

    KE_ROW = TAPS * BW          # 250 elems per (band, row)
    D_ROW = C * DW              # 42 elems per (band, row)

    with tile.TileContext(nc) as tc:
        with tc.tile_pool(name="const", bufs=1) as cpool, \
             tc.tile_pool(name="kt", bufs=2) as kpool, \
             tc.tile_pool(name="et", bufs=2) as epool, \
             tc.tile_pool(name="dt", bufs=1) as dpool, \
             tc.tile_pool(name="qt", bufs=4) as qpool, \
             tc.tile_pool(name="rt", bufs=2) as rpool, \
             tc.tile_pool(name="ot", bufs=2) as opool, \
             tc.tile_pool(name="ps", bufs=2, space="PSUM") as ppool:

            id_sb = cpool.tile([NB, NB], f16)
            nc.sync.dma_start(out=id_sb[:], in_=d_id.ap())

            D0 = dpool.tile([NB, DR, C, DW], f16, tag="d0")
            D1 = dpool.tile([NB, DR, C, DW], f16, tag="d1")

            st_eng = [nc.sync, nc.scalar]

            # deferred epilogue: normalize+store of tile t runs while
            # tile t+1's products stream, so the DVE never stalls
            # waiting for the PE to drain the accumulation group
            pending = []

            def ep_act():
                # P: PSUM -> SBUF fp16 on ACT.  Issued BEFORE the next
                # exp so it isn't stuck behind a 9.7us ACT op when the
                # DVE-side normalize needs it.
                if not pending:
                    return
                ent = pending[0]
                t, r0, nr, pacc = ent[0], ent[1], ent[2], ent[3]
                Pst = opool.tile([NB, nr, C, BW], f16, tag="pst")
                p_view = AP(
                    pacc[:].tensor, 0,
                    [[4 * 512, NB], [BW, nr], [512, C], [1, BW]],
                )
                nc.scalar.activation(
                    Pst[:], p_view, mybir.ActivationFunctionType.Copy,
                )
                ent[4] = Pst

            def ep_dve():
                # normalize+store, issued one di-group into the next
                # tile so the PSUM-stop semaphores are already clear
                if not pending:
                    return
                t, r0, nr, pacc, Pst = pending.pop(0)
                fd = nr * BW
                R32 = rpool.tile([NB, nr, BW], f32, tag="r32")
                nc.vector.reciprocal_approx_fast(
                    out=R32[:].rearrange("p r x -> p (r x)"),
                    in_=pacc[:, 3, 0:fd],
                )
                R16 = rpool.tile([NB, nr, BW], f16, tag="r16")
                nc.vector.tensor_copy(
                    R16[:].rearrange("p r x -> p (r x)"),
                    R32[:].rearrange("p r x -> p (r x)"),
                )
                outst = opool.tile([NB, nr, C, BW], f16, tag="o")
                r_bc = AP(
                    R16[:].tensor, 0,
                    [[nr * BW, NB], [BW, nr], [0, C], [1, BW]],
                )
                nc.vector.tensor_tensor(
                    outst[:], Pst[:], r_bc, mybir.AluOpType.mult)
                st_eng[t % 2].dma_start(
                    out=d_out.ap()[:, r0:r0 + nr],
                    in_=outst[:].rearrange("p r c x -> p r (c x)"),
                )

            for t, (r0, nr) in enumerate(TILES):
                fd = nr * BW
                kE = kpool.tile([NB, nr, KE_ROW], f16, tag="ke")
                nc.gpsimd.dma_start(
                    out=kE[:], in_=d_ke.ap()[:, r0:r0 + nr]
                )
                # D chunks ride the same SWDGE queue, behind the kE
                # loads whose consumers they race
                if t == 0:
                    nc.gpsimd.dma_start(
                        out=D0[:, 0:DSPLIT], in_=d_db0.ap()[:, 0:DSPLIT])
                    nc.gpsimd.dma_start(
                        out=D1[:, 0:DSPLIT], in_=d_db1.ap()[:, 0:DSPLIT])
                elif t == 2:
                    # rows >= 188 are first needed by tile 4; keep these
                    # big chunks behind tile 2's kE load in the SWDGE queue
                    nc.gpsimd.dma_start(
                        out=D0[:, DSPLIT:DR], in_=d_db0.ap()[:, DSPLIT:DR])
                    nc.gpsimd.dma_start(
                        out=D1[:, DSPLIT:DR], in_=d_db1.ap()[:, DSPLIT:DR])

                ep_act()
                E = epool.tile([NB, nr, TAPS, BW], f16, tag="e")
                nc.scalar.activation(
                    E[:].rearrange("p r t x -> p (r t x)"),
                    kE[:].rearrange("p r k -> p (r k)"),
                    mybir.ActivationFunctionType.Exp,
                )
                eap = E[:]

                pacc = ppool.tile([NB, 4, 512], f32, tag="pacc")

                for di in range(KW):
                    # products: q = E * D, dj batched in the outer free dim
                    qes, qos = [], []
                    for c in range(C):
                        qe = qpool.tile([NB, 3, nr, BW], f16, tag="qe")
                        qo = qpool.tile([NB, 2, nr, BW], f16, tag="qo")
                        e_even = AP(
                            eap.tensor, (KW * di) * BW,
                            [[nr * KE_ROW, NB], [BW, 3], [KE_ROW, nr], [1, BW]],
                        )
                        e_odd = AP(
                            eap.tensor, (KW * di + 3) * BW,
                            [[nr * KE_ROW, NB], [BW, 2], [KE_ROW, nr], [1, BW]],
                        )
                        doff = (r0 + di) * D_ROW + c * DW
                        d_even = AP(
                            D0[:].tensor, doff,
                            [[DR * D_ROW, NB], [2, 3], [D_ROW, nr], [1, BW]],
                        )
                        d_odd = AP(
                            D1[:].tensor, doff,
                            [[DR * D_ROW, NB], [2, 2], [D_ROW, nr], [1, BW]],
                        )
                        nc.vector.tensor_tensor(
                            qe[:], e_even, d_even, mybir.AluOpType.mult)
                        qo_eng = nc.gpsimd if (di, c) in GP_QO else nc.vector
                        qo_eng.tensor_tensor(
                            qo[:], e_odd, d_odd, mybir.AluOpType.mult)
                        qes.append(qe)
                        qos.append(qo)
                    # sumexp: 5 identity matmuls straight off E (no DVE dep)
                    for k in range(KW):
                        tp = KW * di + k
                        nc.tensor.matmul(
                            out=pacc[:, 3, 0:fd],
                            lhsT=id_sb[:],
                            rhs=eap[:, :, tp, :],
                            start=(tp == 0),
                            stop=(tp == TAPS - 1),
                        )
                    # tap accumulation
                    for c in range(C):
                        for k in range(3):
                            nc.tensor.matmul(
                                out=pacc[:, c, 0:fd],
                                lhsT=id_sb[:],
                                rhs=qes[c][:, k],
                                start=(di == 0 and k == 0),
                                stop=False,
                            )
                        for k in range(2):
                            nc.tensor.matmul(
                                out=pacc[:, c, 0:fd],
                                lhsT=id_sb[:],
                                rhs=qos[c][:, k],
                                start=False,
                                stop=(di == KW - 1 and k == 1),
                            )
                    if di == 0:
                        ep_dve()

                pending.append([t, r0, nr, pacc, None])

            ep_act()
            ep_dve()

    nc.compile()
    return nc


def get_program():
    if "nc" not in _CACHE:
        _CACHE["nc"] = _build_program()
    return _CACHE["nc"]


def make_shards(data: np.ndarray, kernels: np.ndarray):
    """Full inputs -> per-core input maps (band layout, fp16)."""
    data = np.asarray(data, dtype=np.float32)
    kernels = np.asarray(kernels, dtype=np.float32)

    kf = kernels[:, TAP_PERM].astype(np.float16)      # [B, 25, H, W]
    dpad = np.zeros((B, C, H + 4, W + 6), dtype=np.float16)
    dpad[:, :, 2:H + 2, 2:W + 2] = data

    in_maps = []
    for core in range(NCORES):
        b, hh = divmod(core, 2)
        r0 = hh * HS
        ks = kf[b, :, r0:r0 + HS, :]                  # [25, 360, 1280]
        ke = np.ascontiguousarray(
            ks.reshape(TAPS, HS, NB, BW).transpose(2, 1, 0, 3)
        ).reshape(NB, HS, TAPS * BW)
        dsl = dpad[b, :, r0:r0 + DR, :]               # [3, 364, 1286]
        win = sliding_window_view(dsl, DW, axis=2)    # [3, 364, 1273, 14]
        db0 = np.ascontiguousarray(
            win[:, :, 0:NB * BW:BW].transpose(2, 1, 0, 3))   # [128,364,3,14]
        db1 = np.ascontiguousarray(
            win[:, :, 1:NB * BW + 1:BW].transpose(2, 1, 0, 3))
        in_maps.append({"ke": ke, "db0": db0, "db1": db1})
    return in_maps


def unshard_out(arr: np.ndarray) -> np.ndarray:
    """Per-core out [NB, HS, C*BW] fp16 -> [C, HS, W] f32."""
    o = arr.reshape(NB, HS, C, BW).transpose(2, 1, 0, 3)
    return np.ascontiguousarray(o).reshape(C, HS, W).astype(np.float32)


def assemble(results) -> np.ndarray:
    out = np.empty((B, C, H, W), dtype=np.float32)
    for core in range(NCORES):
        b, hh = divmod(core, 2)
        out[b, :, hh * HS:(hh + 1) * HS, :] = unshard_out(results[core]["out"])
    return out


def kernel(data: np.ndarray, kernels: np.ndarray) -> np.ndarray:
    from concourse.bass_utils import run_bass_kernel_spmd

    nc = get_program()
    in_maps = make_shards(data, kernels)
    res = run_bass_kernel_spmd(nc, in_maps, list(range(NCORES)))
    return assemble(res.results)


if __name__ == "__main__":
    get_program()
    print("program built OK")
